# revision 19
# baseline (speedup 1.0000x reference)
"""Trainium2 Bass kernel for nn_ClassicalHybridClassifier (v2).

Pipeline: conv1(5x5,s2) -> maxpool(2,s1) -> conv2(3x3,s2) -> maxpool(2,s1)
          -> fc1 [120,55815] -> fc2 -> fc3 -> qnn tanh stack -> RBF vs 8192
          train states -> [1,2] output.

Sharding: each of the 8 cores computes a horizontal band of the conv pipeline
(bands over the 61 pool2 output rows: 8,8,8,8,8,7,7,7) and the matching
contraction slice of fc1 (tensor-parallel over fc1's 55815 input dim). One
AllReduce of the [10,120] fc1 partials; the tiny tail (fc2/fc3/qnn/RBF over
all 8192 train states) is replicated on every core.

v2 vs v1:
- whole conv/fc1 path in bf16 (measured end-to-end rel err ~3e-3 vs 2e-2 gate)
- conv1 contraction packed into a single K=128 matmul group (c0,c1 full 43
  rows; c2 rows 0..41) plus a K=5 fixup matmul for the (c2,r42) row whose
  kx taps are materialized host-side as 5 shifted stream rows
- conv biases folded into the PSUM-eviction activation (no ones rows)
- fc1 single-bf16 (61 matmuls), j-major pool2 so fc1 follows immediately
- chunked, prioritized input DMAs (weights-early, 3 x chunks) on HWDGE so the
  PE starts within ~3us and stays p-state hot (warmup matmul train up front)
- dummy AllReduce early to absorb the CC engine's ~11us cold-start before the
  real AllReduce
- tail: PE broadcast of fs, RBF in inner-product form with exp(-|ts|^2)
  folded host-side into the classifier weights
"""

import numpy as np
import ml_dtypes

import concourse.bass as bass
import concourse.mybir as mybir
import concourse.tile as tile
from concourse import bass_utils, bacc

F32 = mybir.dt.float32
BF16 = mybir.dt.bfloat16
AF = mybir.ActivationFunctionType
ALU = mybir.AluOpType
AX = mybir.AxisListType

N_CORES = 8
BANDS = [(0, 8), (8, 16), (16, 24), (24, 32), (32, 40), (40, 47), (47, 54), (54, 61)]

B = 10          # batch
XR = 43         # x rows per core (padded)
XC = 252        # x cols incl 1+1 zero pad
C1R = 20        # conv1 out rows per core (padded)
P1R = 19        # pool1 rows per core (padded)
C2R = 9         # conv2 out rows per core (padded)
NJ = 61         # pool2 / fc1 spatial columns

C1_CHUNKS = [(0, 4), (4, 3), (7, 3)]     # conv1/pool1 image chunks (ni*124<=512)
C2_CHUNKS = [(0, 8), (8, 2)]             # conv2 image chunks (ni*62<=512)
J_BLOCKS = [(0, 31), (31, 30)]           # pool2/fc1 j blocks (B*jw<=512)


def _build_nc():
    nc = bacc.Bacc("TRN2", target_bir_lowering=False, debug=False,
                   num_devices=N_CORES)

    d = {}
    def din(name, shape, dt):
        d[name] = nc.dram_tensor(name, list(shape), dt, kind="ExternalInput").ap()

    din("we", (128, 1480), BF16)       # w1|w1fix|s1m|s2a|s2b|w2f
    din("bsmall", (128, 4), F32)       # conv biases
    din("xs", (128, B * XC), BF16)     # x band, K=128 partition layout
    din("xf", (5, B * 124), BF16)      # (c2,r42) kx-shifted fixup stream rows
    din("wslab", (120, NJ, 120), BF16) # fc1 weight slab
    din("pack1", (128, 576), F32)      # tail smalls

    out_d = nc.dram_tensor("out", [1, 2], F32, kind="ExternalOutput").ap()
    warm_d = nc.dram_tensor("warm", [1, 4], F32, kind="ExternalOutput").ap()

    with tile.TileContext(nc) as tc:
        with (
            tc.tile_pool(name="sb", bufs=1) as sb,
            tc.tile_pool(name="dr", bufs=1, space="DRAM") as dr,
        ):
            # ---- input tiles ----
            we_t = sb.tile([128, 1480], BF16)
            bs_t = sb.tile([128, 4], F32)
            xs_t = sb.tile([128, B * XC], BF16)
            xf_t = sb.tile([5, B * 124], BF16)
            wslab_t = sb.tile([120, NJ, 120], BF16)
            pack1_t = sb.tile([128, 576], F32)

            # ---- warm the CC engine: a near-dependency-free collective fires
            # within a few us, occupies the CC queue at its ~20us wakeup, so
            # the real AllReduce (whose trigger lands while this mesh runs) is
            # processed back-to-back instead of waiting for a CC idle-poll ----
            dums = sb.tile([1, 8], F32)
            nc.vector.memset(dums[:], 0.0)
            dum_in = dr.tile([1, 8], F32)
            dum_out = dr.tile([1, 8], F32, addr_space="Shared")
            nc.gpsimd.dma_start(dum_in[:], dums[:])
            nc.gpsimd.collective_compute(
                "AllReduce", ALU.add,
                replica_groups=[list(range(N_CORES))],
                ins=[dum_in.opt()], outs=[dum_out.opt()])

            # ---- DMAs in. Early tensors (conv weights, x chunks) go on the
            # gpsimd SWDGE queue (10 DMA engines); the big fc1 slab + tail pack
            # go on the sync/scalar HWDGE queues (6 engines) ----
            nc.gpsimd.dma_start(bs_t[:], d["bsmall"][:])
            nc.gpsimd.dma_start(we_t[:, 0:720], d["we"][:, 0:720])
            nc.gpsimd.dma_start(xs_t[:, 0:4 * XC], d["xs"][:, 0:4 * XC])
            nc.gpsimd.dma_start(xf_t[:], d["xf"][:])
            nc.gpsimd.dma_start(xs_t[:, 4 * XC:7 * XC], d["xs"][:, 4 * XC:7 * XC])
            nc.gpsimd.dma_start(xs_t[:, 7 * XC:B * XC], d["xs"][:, 7 * XC:B * XC])
            nc.gpsimd.dma_start(we_t[:, 720:1480], d["we"][:, 720:1480])
            nc.gpsimd.dma_start(wslab_t[:, 0:31, :], d["wslab"][:, 0:31, :])
            nc.scalar.dma_start(wslab_t[:, 31:NJ, :], d["wslab"][:, 31:NJ, :])
            nc.sync.dma_start(pack1_t[:], d["pack1"][:])

            x_r = xs_t[:].rearrange("p (i c) -> p i c", c=XC)
            xf_r = xf_t[:].rearrange("p (i c) -> p i c", c=124)
            w1_t = we_t[:, 0:600]                  # [128, 5*120]
            w1fix_t = we_t[0:5, 600:720]           # [5, 120] (cols 114:120 live)
            s1m_t = we_t[0:120, 720:834]           # [120, 114]
            s2a_t = we_t[0:120, 834:954]           # [120, 120]
            s2b_t = we_t[0:15, 954:1074]           # [15, 120]
            w2f_t = we_t[0:114, 1074:1479]         # [114, 3*135]
            b1 = bs_t[0:120, 0:1]
            b2a = bs_t[0:120, 1:2]
            b2b = bs_t[0:15, 2:3]

            small = {
                "fc1b": pack1_t[0:120, 0:1],
                "w2fcT": pack1_t[0:120, 1:85],
                "fc2b": pack1_t[0:84, 85:86],
                "w3fcT": pack1_t[0:84, 86:87],
                "b3vec": pack1_t[0:B, 87:88],
                "wq1T": pack1_t[0:B, 88:108],
                "wq2T": pack1_t[0:20, 108:113],
                "idt10": pack1_t[0:B, 113:123],
                "kclsb": pack1_t[0:1, 123:125],
                "ts2_r": pack1_t[:, 128:448].rearrange("p (a b) -> p a b", b=5),
                "acls_r": pack1_t[:, 448:576].rearrange("p (a b) -> p a b", b=64),
            }

            # ---- PE warmup (p-state ramp) during input DMA ----
            with tc.tile_pool(name="ps_w", bufs=1, space="PSUM") as ps_w:
                wsc = sb.tile([128, 512], BF16)
                nc.vector.memset(wsc[:], 0.0)
                wps = ps_w.tile([128, 512], F32)
                for i in range(10):
                    nc.tensor.matmul(wps[:], wsc[:, 0:128], wsc[:],
                                     start=(i == 0), stop=(i == 9))
                wout = sb.tile([1, 4], F32)
                nc.vector.tensor_copy(wout[:], wps[0:1, 0:4])
                nc.sync.dma_start(warm_d[:], wout[:])

            # ---- conv1 + pool1 ----
            Cs = sb.tile([120, B, 124], BF16)      # conv1 evict (relu+bias)
            Ch = sb.tile([120, B, 123], BF16)      # horizontal max
            V = sb.tile([114, B, 125], BF16)       # pool1 out
            nc.vector.memset(V[:, :, 0:1], 0.0)
            nc.vector.memset(V[:, :, 124:125], 0.0)

            with tc.tile_pool(name="ps_1", bufs=1, space="PSUM") as ps1:
                Cp = ps1.tile([120, 1536], F32)    # conv1 psum, 3 banks
                Sh = ps1.tile([114, 1536], F32)    # pool1 vertical-shift psum

                def conv1_chunk(ci, i0, ni):
                    n = ni * 124
                    for kx in range(5):
                        nc.tensor.matmul(
                            Cp[:, ci * 512: ci * 512 + n],
                            w1_t[:, kx * 120:(kx + 1) * 120],
                            x_r[:, i0:i0 + ni, kx:kx + 248:2],
                            start=(kx == 0), stop=False)
                    nc.tensor.matmul(
                        Cp[:, ci * 512: ci * 512 + n],
                        w1fix_t[:], xf_r[:, i0:i0 + ni, :],
                        start=False, stop=True)

                def conv1_evict(ci, i0, ni):
                    cv = Cp[:, ci * 512: ci * 512 + ni * 124].rearrange(
                        "p (i x) -> p i x", x=124)
                    nc.scalar.activation(Cs[:, i0:i0 + ni, :], cv, AF.Relu,
                                         bias=b1)
                    nc.vector.tensor_max(Ch[:, i0:i0 + ni, :],
                                         Cs[:, i0:i0 + ni, 0:123],
                                         Cs[:, i0:i0 + ni, 1:124])

                def pool1_chunk(ci, i0, ni):
                    n = ni * 123
                    nc.tensor.matmul(
                        Sh[:, ci * 512: ci * 512 + n],
                        s1m_t[:, 0:114], Ch[:, i0:i0 + ni, :],
                        start=True, stop=True)
                    sv = Sh[:, ci * 512: ci * 512 + n].rearrange(
                        "p (i x) -> p i x", x=123)
                    nc.vector.tensor_max(V[0:114, i0:i0 + ni, 1:124],
                                         Ch[0:114, i0:i0 + ni, :], sv)

                # PE order keeps the systolic array continuously busy
                conv1_chunk(0, 0, 4)
                conv1_evict(0, 0, 4)
                conv1_chunk(1, 4, 3)
                conv1_evict(1, 4, 3)
                pool1_chunk(0, 0, 4)
                conv1_chunk(2, 7, 3)
                conv1_evict(2, 7, 3)
                pool1_chunk(1, 4, 3)
                pool1_chunk(2, 7, 3)

            # ---- conv2 + pool2 + fc1 ----
            C2s_a = sb.tile([120, B, 62], BF16)
            C2s_b = sb.tile([15, B, 62], BF16)
            C2h_a = sb.tile([120, B, 61], BF16)
            C2h_b = sb.tile([15, B, 61], BF16)
            V2 = sb.tile([120, B, NJ], BF16)       # pool2 out
            fc1s = sb.tile([B, 120], F32)

            with tc.tile_pool(name="ps_2", bufs=1, space="PSUM") as ps2:
                C2a = ps2.tile([120, 1024], F32)
                C2b = ps2.tile([15, 1024], F32)
                Sh2 = ps2.tile([120, 1024], F32)
                fps = ps2.tile([B, 120], F32)

                def conv2_chunk(ci, i0, ni):
                    n = ni * 62
                    for grp, (cp, m0, m1) in enumerate(
                            ((C2a, 0, 120), (C2b, 120, 135))):
                        for kxp in range(3):
                            nc.tensor.matmul(
                                cp[:, ci * 512: ci * 512 + n],
                                w2f_t[:, kxp * 135 + m0: kxp * 135 + m1],
                                V[:, i0:i0 + ni, kxp:kxp + 123:2],
                                start=(kxp == 0), stop=(kxp == 2))

                def conv2_evict(ci, i0, ni):
                    for cp, cs, ch, bb in ((C2a, C2s_a, C2h_a, b2a),
                                           (C2b, C2s_b, C2h_b, b2b)):
                        cv = cp[:, ci * 512: ci * 512 + ni * 62].rearrange(
                            "p (i x) -> p i x", x=62)
                        nc.scalar.activation(cs[:, i0:i0 + ni, :], cv, AF.Relu,
                                             bias=bb)
                        nc.vector.tensor_max(ch[:, i0:i0 + ni, :],
                                             cs[:, i0:i0 + ni, 0:61],
                                             cs[:, i0:i0 + ni, 1:62])

                def pool2_block(bi, j0, jw):
                    n = jw * B
                    nc.tensor.matmul(
                        Sh2[:, bi * 512: bi * 512 + n],
                        s2a_t[:], C2h_a[:, :, j0:j0 + jw],
                        start=True, stop=False)
                    nc.tensor.matmul(
                        Sh2[:, bi * 512: bi * 512 + n],
                        s2b_t[:], C2h_b[:, :, j0:j0 + jw],
                        start=False, stop=True)
                    sv = Sh2[:, bi * 512: bi * 512 + n].rearrange(
                        "p (i j) -> p i j", j=jw)
                    nc.vector.tensor_max(V2[:, :, j0:j0 + jw],
                                         C2h_a[:, :, j0:j0 + jw], sv)

                conv2_chunk(0, 0, 8)
                conv2_evict(0, 0, 8)
                conv2_chunk(1, 8, 2)
                conv2_evict(1, 8, 2)
                pool2_block(0, 0, 31)
                pool2_block(1, 31, 30)

                for j in range(NJ):
                    nc.tensor.matmul(fps[:], V2[:, :, j], wslab_t[:, j, :],
                                     start=(j == 0), stop=(j == NJ - 1))
                nc.vector.tensor_copy(fc1s[:], fps[:])

            # ---- AllReduce fc1 partials ----
            arin = dr.tile([B, 120], F32)
            arout = dr.tile([B, 120], F32, addr_space="Shared")
            nc.sync.dma_start(arin[:], fc1s[:])
            nc.gpsimd.collective_compute(
                "AllReduce", ALU.add,
                replica_groups=[list(range(N_CORES))],
                ins=[arin.opt()], outs=[arout.opt()])
            h1post = sb.tile([B, 120], F32)
            nc.sync.dma_start(h1post[:], arout[:])

            # ---- tail (replicated) ----
            h1 = sb.tile([120, B], F32)
            h2 = sb.tile([84, B], F32)
            h10 = sb.tile([B, 1], F32)
            s1 = sb.tile([20, 1], F32)
            fs_row = sb.tile([1, 5], F32)
            fsb_sb = sb.tile([128, 5], F32)
            fs2 = sb.tile([128, 5], F32)
            negs2 = sb.tile([128, 1], F32)
            prodz = sb.tile([128, 64, 5], F32)
            z = sb.tile([128, 64], F32)
            kxv = sb.tile([128, 64], F32)
            pr = sb.tile([128, 2, 64], F32)
            krw = sb.tile([128, 2], F32)
            ones_row = sb.tile([1, 128], F32)
            ones_col = sb.tile([128, 1], F32)
            out_sb = sb.tile([1, 2], F32)
            nc.vector.memset(ones_row[:], 1.0)
            nc.vector.memset(ones_col[:], 1.0)

            with tc.tile_pool(name="ps_4", bufs=1, space="PSUM") as ps4:
                tp = ps4.tile([120, B], F32)
                nc.tensor.transpose(tp[:], h1post[:], small["idt10"][:])
                nc.scalar.activation(h1[:], tp[:], AF.Relu,
                                     bias=small["fc1b"][:])

                p2 = ps4.tile([84, B], F32)
                nc.tensor.matmul(p2[:], small["w2fcT"][:], h1[:],
                                 start=True, stop=True)
                nc.scalar.activation(h2[:], p2[:], AF.Relu,
                                     bias=small["fc2b"][:])

                p3 = ps4.tile([B, 1], F32)
                nc.tensor.matmul(p3[:], h2[:], small["w3fcT"][:],
                                 start=True, stop=True)
                nc.scalar.activation(h10[:], p3[:], AF.Identity,
                                     bias=small["b3vec"][:])

                p4 = ps4.tile([20, 1], F32)
                nc.tensor.matmul(p4[:], small["wq1T"][:], h10[:],
                                 start=True, stop=True)
                nc.scalar.activation(s1[:], p4[:], AF.Tanh)

                p5 = ps4.tile([1, 5], F32)
                nc.tensor.matmul(p5[:], s1[:], small["wq2T"][:],
                                 start=True, stop=True)
                nc.scalar.activation(fs_row[:], p5[:], AF.Tanh)

                # broadcast fs to 128 partitions on the PE
                fsb_ps = ps4.tile([128, 5], F32)
                nc.tensor.matmul(fsb_ps[:], ones_row[:], fs_row[:],
                                 start=True, stop=True)
                nc.scalar.activation(fsb_sb[:], fsb_ps[:], AF.Identity)

                # |fs|^2 -> negs2 ; z = 2<ts, fs> ; K = exp(z - |fs|^2)
                nc.vector.scalar_tensor_tensor(
                    fs2[:], fsb_sb[:], -1.0, fsb_sb[:],
                    op0=ALU.mult, op1=ALU.mult)
                nc.vector.reduce_sum(negs2[:], fs2[:], axis=AX.X)
                nc.vector.tensor_mul(
                    prodz[:], small["ts2_r"][:],
                    fsb_sb[:].unsqueeze(1).broadcast_to([128, 64, 5]))
                nc.vector.reduce_sum(z[:], prodz[:], axis=AX.X)
                nc.scalar.activation(kxv[:], z[:], AF.Exp, bias=negs2[:])
                nc.vector.tensor_mul(
                    pr[:], small["acls_r"][:],
                    kxv[:].unsqueeze(1).broadcast_to([128, 2, 64]))
                nc.vector.reduce_sum(krw[:], pr[:], axis=AX.X)

                p6 = ps4.tile([1, 2], F32)
                nc.tensor.matmul(p6[:], ones_col[:], krw[:],
                                 start=True, stop=True)
                nc.vector.tensor_add(out_sb[:], p6[:], small["kclsb"][:])

            nc.sync.dma_start(out_d[:], out_sb[:])

    nc.compile()
    return nc


def _prep_inputs(inputs):
    f32 = np.float32
    bf16 = ml_dtypes.bfloat16
    x = np.asarray(inputs["x"], f32)
    conv1_w = np.asarray(inputs["conv1_w"], f32)
    conv1_b = np.asarray(inputs["conv1_b"], f32)
    conv2_w = np.asarray(inputs["conv2_w"], f32)
    conv2_b = np.asarray(inputs["conv2_b"], f32)
    fc1_w = np.asarray(inputs["fc1_w"], f32)
    fc1_b = np.asarray(inputs["fc1_b"], f32)
    fc2_w = np.asarray(inputs["fc2_w"], f32)
    fc2_b = np.asarray(inputs["fc2_b"], f32)
    fc3_w = np.asarray(inputs["fc3_w"], f32)
    fc3_b = np.asarray(inputs["fc3_b"], f32)
    qnn_w1 = np.asarray(inputs["qnn_w1"], f32)
    qnn_w2 = np.asarray(inputs["qnn_w2"], f32)
    ts = np.asarray(inputs["train_states"], f32)
    kcls_w = np.asarray(inputs["kcls_w"], f32)
    kcls_b = np.asarray(inputs["kcls_b"], f32)

    pack1 = np.zeros((128, 576), f32)
    pack1[0:120, 0:1] = fc1_b.reshape(120, 1)
    pack1[0:120, 1:85] = fc2_w.T
    pack1[0:84, 85:86] = fc2_b.reshape(84, 1)
    pack1[0:84, 86:87] = fc3_w.T
    pack1[0:B, 87:88] = fc3_b[0]
    pack1[0:B, 88:108] = qnn_w1.T
    pack1[0:20, 108:113] = qnn_w2.T
    pack1[0:B, 113:123] = np.eye(B, dtype=f32)
    pack1[0:1, 123:125] = kcls_b.reshape(1, 2)
    pack1[:, 128:448] = (2.0 * ts).reshape(128, 320)
    acls = kcls_w * np.exp(-np.sum(ts * ts, axis=-1))[None, :]   # [2, 8192]
    pack1[:, 448:576] = acls.reshape(2, 128, 64).transpose(1, 0, 2).reshape(128, 128)

    bsmall = np.zeros((128, 4), f32)
    bsmall[0:120, 0] = np.tile(conv1_b, C1R)
    bsmall[0:120, 1] = np.tile(conv2_b, 8)
    bsmall[0:15, 2] = conv2_b

    fc1_w4 = fc1_w.reshape(120, 15, 61, 61)

    in_maps = []
    for a, b in BANDS:
        nb = b - a
        Y0 = 2 * a - 1          # conv1 row of y_loc 0 (also pool1 row of py_loc 0)
        X0 = 4 * a - 3          # x row of r_loc 0

        # x slab: K=128 partitions = c0 r0-42 | c1 r0-42 | c2 r0-41
        xsl = np.zeros((3, XR, B, XC), f32)
        r_lo, r_hi = max(0, X0), min(250, X0 + XR)
        xsl[:, r_lo - X0: r_hi - X0, :, 1:251] = (
            x[:, :, r_lo:r_hi, :].transpose(1, 2, 0, 3))
        x128 = np.concatenate([xsl[0], xsl[1], xsl[2][0:42]], axis=0)
        xf = np.zeros((5, B, 124), f32)
        for kx in range(5):
            xf[kx] = xsl[2][42][:, kx:kx + 248:2]

        # conv1 banded weights: K=(c, r_loc), M=(y_loc, och), per kx
        w1 = np.zeros((3, XR, 5, 120), f32)
        for y_loc in range(C1R):
            y = Y0 + y_loc
            if not (0 <= y <= 123):
                continue
            for ky in range(5):
                r_loc = 2 * y_loc + ky
                if r_loc >= XR:
                    continue
                for c in range(3):
                    w1[c, r_loc, :, y_loc * 6: y_loc * 6 + 6] = \
                        conv1_w[:, c, ky, :].T
        w128 = np.concatenate([w1[0], w1[1], w1[2][0:42]], axis=0)  # [128,5,120]
        w1fix = np.zeros((5, 6), f32)
        if 0 <= Y0 + 19 <= 123:
            w1fix[:, :] = conv1_w[:, 2, 4, :].T     # (c2, ky=4) taps, [kx, och]

        # conv2 banded weights: K=(py_loc, ich), M=(i2_loc, och2)
        w2 = np.zeros((114, 3, 135), f32)
        for i2_loc in range(C2R):
            i2 = a + i2_loc
            if i2 > 61:
                continue
            for kyp in range(3):
                py_loc = 2 * i2_loc + kyp
                py = Y0 + py_loc
                if py_loc >= P1R or not (0 <= py <= 122):
                    continue
                for ich in range(6):
                    q = py_loc * 6 + ich
                    m0 = i2_loc * 15
                    w2[q, :, m0:m0 + 15] = conv2_w[:, ich, kyp, :].T

        # partition-shift matrices
        s1m = np.zeros((120, 114), f32)
        for m in range(114):
            s1m[m + 6, m] = 1.0
        s2a = np.zeros((120, 120), f32)
        s2b = np.zeros((15, 120), f32)
        for m in range(105):
            s2a[m + 15, m] = 1.0
        for m in range(105, 120):
            s2b[m - 105, m] = 1.0

        # fc1 weight slab: [p=(i2_loc,och2), j, och1]
        wsl = np.zeros((8, 15, NJ, 120), f32)
        nrow = min(nb, 8)
        wsl[0:nrow] = fc1_w4[:, :, a:a + nrow, :].transpose(2, 1, 3, 0)
        wslab = wsl.reshape(120, NJ, 120).astype(bf16)

        we = np.zeros((128, 1480), f32)
        we[:, 0:600] = w128.reshape(128, 600)
        we[0:5, 714:720] = w1fix        # M cols 114..119 of the [5,120] block
        we[0:120, 720:834] = s1m
        we[0:120, 834:954] = s2a
        we[0:15, 954:1074] = s2b
        we[0:114, 1074:1479] = w2.reshape(114, 405)

        m = {
            "we": we.astype(bf16),
            "bsmall": bsmall,
            "xs": x128.reshape(128, B * XC).astype(bf16),
            "xf": xf.reshape(5, B * 124).astype(bf16),
            "wslab": np.ascontiguousarray(wslab),
            "pack1": pack1,
        }
        in_maps.append(m)
    return in_maps


_NC_CACHE = None


def kernel(**inputs) -> np.ndarray:
    global _NC_CACHE
    if _NC_CACHE is None:
        _NC_CACHE = _build_nc()
    nc = _NC_CACHE
    in_maps = _prep_inputs(inputs)
    res = bass_utils.run_bass_kernel_spmd(
        nc, in_maps, core_ids=list(range(N_CORES)))
    return res.results[0]["out"]


# revision 20
# speedup vs baseline: 1.1185x; 1.1185x over previous
"""Trainium2 Bass kernel for nn_ClassicalHybridClassifier (v2).

Pipeline: conv1(5x5,s2) -> maxpool(2,s1) -> conv2(3x3,s2) -> maxpool(2,s1)
          -> fc1 [120,55815] -> fc2 -> fc3 -> qnn tanh stack -> RBF vs 8192
          train states -> [1,2] output.

Sharding: each of the 8 cores computes a horizontal band of the conv pipeline
(bands over the 61 pool2 output rows: 8,8,8,8,8,7,7,7) and the matching
contraction slice of fc1 (tensor-parallel over fc1's 55815 input dim). One
AllReduce of the [10,120] fc1 partials; the tiny tail (fc2/fc3/qnn/RBF over
all 8192 train states) is replicated on every core.

v2 vs v1:
- whole conv/fc1 path in bf16 (measured end-to-end rel err ~3e-3 vs 2e-2 gate)
- conv1 contraction packed into a single K=128 matmul group (c0,c1 full 43
  rows; c2 rows 0..41) plus a K=5 fixup matmul for the (c2,r42) row whose
  kx taps are materialized host-side as 5 shifted stream rows
- conv biases folded into the PSUM-eviction activation (no ones rows)
- fc1 single-bf16 (61 matmuls), j-major pool2 so fc1 follows immediately
- chunked, prioritized input DMAs (weights-early, 3 x chunks) on HWDGE so the
  PE starts within ~3us and stays p-state hot (warmup matmul train up front)
- dummy AllReduce early to absorb the CC engine's ~11us cold-start before the
  real AllReduce
- tail: PE broadcast of fs, RBF in inner-product form with exp(-|ts|^2)
  folded host-side into the classifier weights
"""

import numpy as np
import ml_dtypes

import concourse.bass as bass
import concourse.mybir as mybir
import concourse.tile as tile
from concourse import bass_utils, bacc

F32 = mybir.dt.float32
BF16 = mybir.dt.bfloat16
AF = mybir.ActivationFunctionType
ALU = mybir.AluOpType
AX = mybir.AxisListType

N_CORES = 8
BANDS = [(0, 8), (8, 16), (16, 24), (24, 32), (32, 40), (40, 47), (47, 54), (54, 61)]

B = 10          # batch
XR = 43         # x rows per core (padded)
XC = 252        # x cols incl 1+1 zero pad
C1R = 20        # conv1 out rows per core (padded)
P1R = 19        # pool1 rows per core (padded)
C2R = 9         # conv2 out rows per core (padded)
NJ = 61         # pool2 / fc1 spatial columns

C1_CHUNKS = [(0, 4), (4, 3), (7, 3)]     # conv1/pool1 image chunks (ni*124<=512)
C2_CHUNKS = [(0, 8), (8, 2)]             # conv2 image chunks (ni*62<=512)
J_BLOCKS = [(0, 31), (31, 30)]           # pool2/fc1 j blocks (B*jw<=512)


def _build_nc():
    nc = bacc.Bacc("TRN2", target_bir_lowering=False, debug=False,
                   num_devices=N_CORES)

    d = {}
    def din(name, shape, dt):
        d[name] = nc.dram_tensor(name, list(shape), dt, kind="ExternalInput").ap()

    din("we", (128, 1480), BF16)       # w1|w1fix|s1m|s2a|s2b|w2f
    din("bsmall", (128, 4), F32)       # conv biases
    din("xs", (128, B * XC), BF16)     # x band, K=128 partition layout
    din("xf", (5, B * 124), BF16)      # (c2,r42) kx-shifted fixup stream rows
    din("wslab", (120, NJ, 120), BF16) # fc1 weight slab
    din("pack1", (128, 576), F32)      # tail smalls

    out_d = nc.dram_tensor("out", [1, 2], F32, kind="ExternalOutput").ap()
    warm_d = nc.dram_tensor("warm", [1, 4], F32, kind="ExternalOutput").ap()

    with tile.TileContext(nc) as tc:
        with (
            tc.tile_pool(name="sb", bufs=1) as sb,
            tc.tile_pool(name="dr", bufs=1, space="DRAM") as dr,
        ):
            # ---- input tiles ----
            we_t = sb.tile([128, 1480], BF16)
            bs_t = sb.tile([128, 4], F32)
            xs_t = sb.tile([128, B * XC], BF16)
            xf_t = sb.tile([5, B * 124], BF16)
            wslab_t = sb.tile([120, NJ, 120], BF16)
            pack1_t = sb.tile([128, 576], F32)

            # ---- DMAs in. Early tensors (conv weights, x chunks) go on the
            # gpsimd SWDGE queue (10 DMA engines); the big fc1 slab + tail pack
            # go on the sync/scalar HWDGE queues (6 engines) ----
            nc.gpsimd.dma_start(bs_t[:], d["bsmall"][:])
            nc.gpsimd.dma_start(we_t[:, 0:720], d["we"][:, 0:720])
            nc.gpsimd.dma_start(xs_t[:, 0:4 * XC], d["xs"][:, 0:4 * XC])
            nc.gpsimd.dma_start(xf_t[:], d["xf"][:])
            nc.gpsimd.dma_start(xs_t[:, 4 * XC:7 * XC], d["xs"][:, 4 * XC:7 * XC])
            nc.gpsimd.dma_start(xs_t[:, 7 * XC:B * XC], d["xs"][:, 7 * XC:B * XC])
            nc.gpsimd.dma_start(we_t[:, 720:1480], d["we"][:, 720:1480])
            nc.gpsimd.dma_start(wslab_t[:, 0:31, :], d["wslab"][:, 0:31, :])
            nc.scalar.dma_start(wslab_t[:, 31:NJ, :], d["wslab"][:, 31:NJ, :])
            nc.sync.dma_start(pack1_t[:], d["pack1"][:])

            x_r = xs_t[:].rearrange("p (i c) -> p i c", c=XC)
            xf_r = xf_t[:].rearrange("p (i c) -> p i c", c=124)
            w1_t = we_t[:, 0:600]                  # [128, 5*120]
            w1fix_t = we_t[0:5, 600:720]           # [5, 120] (cols 114:120 live)
            s1m_t = we_t[0:120, 720:834]           # [120, 114]
            s2a_t = we_t[0:120, 834:954]           # [120, 120]
            s2b_t = we_t[0:15, 954:1074]           # [15, 120]
            w2f_t = we_t[0:114, 1074:1479]         # [114, 3*135]
            b1 = bs_t[0:120, 0:1]
            b2a = bs_t[0:120, 1:2]
            b2b = bs_t[0:15, 2:3]

            small = {
                "fc1b": pack1_t[0:120, 0:1],
                "w2fcT": pack1_t[0:120, 1:85],
                "fc2b": pack1_t[0:84, 85:86],
                "w3fcT": pack1_t[0:84, 86:87],
                "b3vec": pack1_t[0:B, 87:88],
                "wq1T": pack1_t[0:B, 88:108],
                "wq2T": pack1_t[0:20, 108:113],
                "idt10": pack1_t[0:B, 113:123],
                "kclsb": pack1_t[0:1, 123:125],
                "ts2_r": pack1_t[:, 128:448].rearrange("p (a b) -> p a b", b=5),
                "acls_r": pack1_t[:, 448:576].rearrange("p (a b) -> p a b", b=64),
            }

            # ---- PE warmup (p-state ramp) during input DMA ----
            with tc.tile_pool(name="ps_w", bufs=1, space="PSUM") as ps_w:
                wsc = sb.tile([128, 512], BF16)
                nc.vector.memset(wsc[:], 0.0)
                wps = ps_w.tile([128, 512], F32)
                for i in range(10):
                    nc.tensor.matmul(wps[:], wsc[:, 0:128], wsc[:],
                                     start=(i == 0), stop=(i == 9))
                wout = sb.tile([1, 4], F32)
                nc.vector.tensor_copy(wout[:], wps[0:1, 0:4])
                nc.sync.dma_start(warm_d[:], wout[:])

            # ---- conv1 + pool1 ----
            Cs = sb.tile([120, B, 124], BF16)      # conv1 evict (relu+bias)
            Ch = sb.tile([120, B, 123], BF16)      # horizontal max
            V = sb.tile([114, B, 125], BF16)       # pool1 out
            nc.vector.memset(V[:, :, 0:1], 0.0)
            nc.vector.memset(V[:, :, 124:125], 0.0)

            with tc.tile_pool(name="ps_1", bufs=1, space="PSUM") as ps1:
                Cp = ps1.tile([120, 1536], F32)    # conv1 psum, 3 banks
                Sh = ps1.tile([114, 1536], F32)    # pool1 vertical-shift psum

                def conv1_chunk(ci, i0, ni):
                    n = ni * 124
                    for kx in range(5):
                        nc.tensor.matmul(
                            Cp[:, ci * 512: ci * 512 + n],
                            w1_t[:, kx * 120:(kx + 1) * 120],
                            x_r[:, i0:i0 + ni, kx:kx + 248:2],
                            start=(kx == 0), stop=False)
                    nc.tensor.matmul(
                        Cp[:, ci * 512: ci * 512 + n],
                        w1fix_t[:], xf_r[:, i0:i0 + ni, :],
                        start=False, stop=True)

                def conv1_evict(ci, i0, ni):
                    cv = Cp[:, ci * 512: ci * 512 + ni * 124].rearrange(
                        "p (i x) -> p i x", x=124)
                    nc.scalar.activation(Cs[:, i0:i0 + ni, :], cv, AF.Relu,
                                         bias=b1)
                    nc.vector.tensor_max(Ch[:, i0:i0 + ni, :],
                                         Cs[:, i0:i0 + ni, 0:123],
                                         Cs[:, i0:i0 + ni, 1:124])

                def pool1_chunk(ci, i0, ni):
                    n = ni * 123
                    nc.tensor.matmul(
                        Sh[:, ci * 512: ci * 512 + n],
                        s1m_t[:, 0:114], Ch[:, i0:i0 + ni, :],
                        start=True, stop=True)
                    sv = Sh[:, ci * 512: ci * 512 + n].rearrange(
                        "p (i x) -> p i x", x=123)
                    nc.vector.tensor_max(V[0:114, i0:i0 + ni, 1:124],
                                         Ch[0:114, i0:i0 + ni, :], sv)

                # PE order keeps the systolic array continuously busy
                conv1_chunk(0, 0, 4)
                conv1_evict(0, 0, 4)
                conv1_chunk(1, 4, 3)
                conv1_evict(1, 4, 3)
                pool1_chunk(0, 0, 4)
                conv1_chunk(2, 7, 3)
                conv1_evict(2, 7, 3)
                pool1_chunk(1, 4, 3)
                pool1_chunk(2, 7, 3)

            # ---- conv2 + pool2 + fc1 ----
            C2s_a = sb.tile([120, B, 62], BF16)
            C2s_b = sb.tile([15, B, 62], BF16)
            C2h_a = sb.tile([120, B, 61], BF16)
            C2h_b = sb.tile([15, B, 61], BF16)
            V2 = sb.tile([120, B, NJ], BF16)       # pool2 out
            fc1s = sb.tile([B, 120], F32)

            with tc.tile_pool(name="ps_2", bufs=1, space="PSUM") as ps2:
                C2a = ps2.tile([120, 1024], F32)
                C2b = ps2.tile([15, 1024], F32)
                Sh2 = ps2.tile([120, 1024], F32)
                fps = ps2.tile([B, 120], F32)

                def conv2_chunk(ci, i0, ni):
                    n = ni * 62
                    for grp, (cp, m0, m1) in enumerate(
                            ((C2a, 0, 120), (C2b, 120, 135))):
                        for kxp in range(3):
                            nc.tensor.matmul(
                                cp[:, ci * 512: ci * 512 + n],
                                w2f_t[:, kxp * 135 + m0: kxp * 135 + m1],
                                V[:, i0:i0 + ni, kxp:kxp + 123:2],
                                start=(kxp == 0), stop=(kxp == 2))

                def conv2_evict(ci, i0, ni):
                    for cp, cs, ch, bb in ((C2a, C2s_a, C2h_a, b2a),
                                           (C2b, C2s_b, C2h_b, b2b)):
                        cv = cp[:, ci * 512: ci * 512 + ni * 62].rearrange(
                            "p (i x) -> p i x", x=62)
                        nc.scalar.activation(cs[:, i0:i0 + ni, :], cv, AF.Relu,
                                             bias=bb)
                        nc.vector.tensor_max(ch[:, i0:i0 + ni, :],
                                             cs[:, i0:i0 + ni, 0:61],
                                             cs[:, i0:i0 + ni, 1:62])

                def pool2_block(bi, j0, jw):
                    n = jw * B
                    nc.tensor.matmul(
                        Sh2[:, bi * 512: bi * 512 + n],
                        s2a_t[:], C2h_a[:, :, j0:j0 + jw],
                        start=True, stop=False)
                    nc.tensor.matmul(
                        Sh2[:, bi * 512: bi * 512 + n],
                        s2b_t[:], C2h_b[:, :, j0:j0 + jw],
                        start=False, stop=True)
                    sv = Sh2[:, bi * 512: bi * 512 + n].rearrange(
                        "p (i j) -> p i j", j=jw)
                    nc.vector.tensor_max(V2[:, :, j0:j0 + jw],
                                         C2h_a[:, :, j0:j0 + jw], sv)

                conv2_chunk(0, 0, 8)
                conv2_evict(0, 0, 8)
                conv2_chunk(1, 8, 2)
                conv2_evict(1, 8, 2)
                pool2_block(0, 0, 31)
                pool2_block(1, 31, 30)

                for j in range(NJ):
                    nc.tensor.matmul(fps[:], V2[:, :, j], wslab_t[:, j, :],
                                     start=(j == 0), stop=(j == NJ - 1))
                nc.vector.tensor_copy(fc1s[:], fps[:])

            # ---- AllReduce fc1 partials ----
            arin = dr.tile([B, 120], F32)
            arout = dr.tile([B, 120], F32, addr_space="Shared")
            nc.sync.dma_start(arin[:], fc1s[:])
            nc.gpsimd.collective_compute(
                "AllReduce", ALU.add,
                replica_groups=[list(range(N_CORES))],
                ins=[arin.opt()], outs=[arout.opt()])
            h1post = sb.tile([B, 120], F32)
            nc.sync.dma_start(h1post[:], arout[:])

            # ---- tail (replicated) ----
            h1 = sb.tile([120, B], F32)
            h2 = sb.tile([84, B], F32)
            h10 = sb.tile([B, 1], F32)
            s1 = sb.tile([20, 1], F32)
            fs_row = sb.tile([1, 5], F32)
            fsb_sb = sb.tile([128, 5], F32)
            fs2 = sb.tile([128, 5], F32)
            negs2 = sb.tile([128, 1], F32)
            prodz = sb.tile([128, 64, 5], F32)
            z = sb.tile([128, 64], F32)
            kxv = sb.tile([128, 64], F32)
            pr = sb.tile([128, 2, 64], F32)
            krw = sb.tile([128, 2], F32)
            ones_row = sb.tile([1, 128], F32)
            ones_col = sb.tile([128, 1], F32)
            out_sb = sb.tile([1, 2], F32)
            nc.vector.memset(ones_row[:], 1.0)
            nc.vector.memset(ones_col[:], 1.0)

            with tc.tile_pool(name="ps_4", bufs=1, space="PSUM") as ps4:
                tp = ps4.tile([120, B], F32)
                nc.tensor.transpose(tp[:], h1post[:], small["idt10"][:])
                nc.scalar.activation(h1[:], tp[:], AF.Relu,
                                     bias=small["fc1b"][:])

                p2 = ps4.tile([84, B], F32)
                nc.tensor.matmul(p2[:], small["w2fcT"][:], h1[:],
                                 start=True, stop=True)
                nc.scalar.activation(h2[:], p2[:], AF.Relu,
                                     bias=small["fc2b"][:])

                p3 = ps4.tile([B, 1], F32)
                nc.tensor.matmul(p3[:], h2[:], small["w3fcT"][:],
                                 start=True, stop=True)
                nc.scalar.activation(h10[:], p3[:], AF.Identity,
                                     bias=small["b3vec"][:])

                p4 = ps4.tile([20, 1], F32)
                nc.tensor.matmul(p4[:], small["wq1T"][:], h10[:],
                                 start=True, stop=True)
                nc.scalar.activation(s1[:], p4[:], AF.Tanh)

                p5 = ps4.tile([1, 5], F32)
                nc.tensor.matmul(p5[:], s1[:], small["wq2T"][:],
                                 start=True, stop=True)
                nc.scalar.activation(fs_row[:], p5[:], AF.Tanh)

                # broadcast fs to 128 partitions on the PE
                fsb_ps = ps4.tile([128, 5], F32)
                nc.tensor.matmul(fsb_ps[:], ones_row[:], fs_row[:],
                                 start=True, stop=True)
                nc.scalar.activation(fsb_sb[:], fsb_ps[:], AF.Identity)

                # |fs|^2 -> negs2 ; z = 2<ts, fs> ; K = exp(z - |fs|^2)
                nc.vector.scalar_tensor_tensor(
                    fs2[:], fsb_sb[:], -1.0, fsb_sb[:],
                    op0=ALU.mult, op1=ALU.mult)
                nc.vector.reduce_sum(negs2[:], fs2[:], axis=AX.X)
                nc.vector.tensor_mul(
                    prodz[:], small["ts2_r"][:],
                    fsb_sb[:].unsqueeze(1).broadcast_to([128, 64, 5]))
                nc.vector.reduce_sum(z[:], prodz[:], axis=AX.X)
                nc.scalar.activation(kxv[:], z[:], AF.Exp, bias=negs2[:])
                nc.vector.tensor_mul(
                    pr[:], small["acls_r"][:],
                    kxv[:].unsqueeze(1).broadcast_to([128, 2, 64]))
                nc.vector.reduce_sum(krw[:], pr[:], axis=AX.X)

                p6 = ps4.tile([1, 2], F32)
                nc.tensor.matmul(p6[:], ones_col[:], krw[:],
                                 start=True, stop=True)
                nc.vector.tensor_add(out_sb[:], p6[:], small["kclsb"][:])

            nc.sync.dma_start(out_d[:], out_sb[:])

    nc.compile()
    return nc


def _prep_inputs(inputs):
    f32 = np.float32
    bf16 = ml_dtypes.bfloat16
    x = np.asarray(inputs["x"], f32)
    conv1_w = np.asarray(inputs["conv1_w"], f32)
    conv1_b = np.asarray(inputs["conv1_b"], f32)
    conv2_w = np.asarray(inputs["conv2_w"], f32)
    conv2_b = np.asarray(inputs["conv2_b"], f32)
    fc1_w = np.asarray(inputs["fc1_w"], f32)
    fc1_b = np.asarray(inputs["fc1_b"], f32)
    fc2_w = np.asarray(inputs["fc2_w"], f32)
    fc2_b = np.asarray(inputs["fc2_b"], f32)
    fc3_w = np.asarray(inputs["fc3_w"], f32)
    fc3_b = np.asarray(inputs["fc3_b"], f32)
    qnn_w1 = np.asarray(inputs["qnn_w1"], f32)
    qnn_w2 = np.asarray(inputs["qnn_w2"], f32)
    ts = np.asarray(inputs["train_states"], f32)
    kcls_w = np.asarray(inputs["kcls_w"], f32)
    kcls_b = np.asarray(inputs["kcls_b"], f32)

    pack1 = np.zeros((128, 576), f32)
    pack1[0:120, 0:1] = fc1_b.reshape(120, 1)
    pack1[0:120, 1:85] = fc2_w.T
    pack1[0:84, 85:86] = fc2_b.reshape(84, 1)
    pack1[0:84, 86:87] = fc3_w.T
    pack1[0:B, 87:88] = fc3_b[0]
    pack1[0:B, 88:108] = qnn_w1.T
    pack1[0:20, 108:113] = qnn_w2.T
    pack1[0:B, 113:123] = np.eye(B, dtype=f32)
    pack1[0:1, 123:125] = kcls_b.reshape(1, 2)
    pack1[:, 128:448] = (2.0 * ts).reshape(128, 320)
    acls = kcls_w * np.exp(-np.sum(ts * ts, axis=-1))[None, :]   # [2, 8192]
    pack1[:, 448:576] = acls.reshape(2, 128, 64).transpose(1, 0, 2).reshape(128, 128)

    bsmall = np.zeros((128, 4), f32)
    bsmall[0:120, 0] = np.tile(conv1_b, C1R)
    bsmall[0:120, 1] = np.tile(conv2_b, 8)
    bsmall[0:15, 2] = conv2_b

    fc1_w4 = fc1_w.reshape(120, 15, 61, 61)

    in_maps = []
    for a, b in BANDS:
        nb = b - a
        Y0 = 2 * a - 1          # conv1 row of y_loc 0 (also pool1 row of py_loc 0)
        X0 = 4 * a - 3          # x row of r_loc 0

        # x slab: K=128 partitions = c0 r0-42 | c1 r0-42 | c2 r0-41
        xsl = np.zeros((3, XR, B, XC), f32)
        r_lo, r_hi = max(0, X0), min(250, X0 + XR)
        xsl[:, r_lo - X0: r_hi - X0, :, 1:251] = (
            x[:, :, r_lo:r_hi, :].transpose(1, 2, 0, 3))
        x128 = np.concatenate([xsl[0], xsl[1], xsl[2][0:42]], axis=0)
        xf = np.zeros((5, B, 124), f32)
        for kx in range(5):
            xf[kx] = xsl[2][42][:, kx:kx + 248:2]

        # conv1 banded weights: K=(c, r_loc), M=(y_loc, och), per kx
        w1 = np.zeros((3, XR, 5, 120), f32)
        for y_loc in range(C1R):
            y = Y0 + y_loc
            if not (0 <= y <= 123):
                continue
            for ky in range(5):
                r_loc = 2 * y_loc + ky
                if r_loc >= XR:
                    continue
                for c in range(3):
                    w1[c, r_loc, :, y_loc * 6: y_loc * 6 + 6] = \
                        conv1_w[:, c, ky, :].T
        w128 = np.concatenate([w1[0], w1[1], w1[2][0:42]], axis=0)  # [128,5,120]
        w1fix = np.zeros((5, 6), f32)
        if 0 <= Y0 + 19 <= 123:
            w1fix[:, :] = conv1_w[:, 2, 4, :].T     # (c2, ky=4) taps, [kx, och]

        # conv2 banded weights: K=(py_loc, ich), M=(i2_loc, och2)
        w2 = np.zeros((114, 3, 135), f32)
        for i2_loc in range(C2R):
            i2 = a + i2_loc
            if i2 > 61:
                continue
            for kyp in range(3):
                py_loc = 2 * i2_loc + kyp
                py = Y0 + py_loc
                if py_loc >= P1R or not (0 <= py <= 122):
                    continue
                for ich in range(6):
                    q = py_loc * 6 + ich
                    m0 = i2_loc * 15
                    w2[q, :, m0:m0 + 15] = conv2_w[:, ich, kyp, :].T

        # partition-shift matrices
        s1m = np.zeros((120, 114), f32)
        for m in range(114):
            s1m[m + 6, m] = 1.0
        s2a = np.zeros((120, 120), f32)
        s2b = np.zeros((15, 120), f32)
        for m in range(105):
            s2a[m + 15, m] = 1.0
        for m in range(105, 120):
            s2b[m - 105, m] = 1.0

        # fc1 weight slab: [p=(i2_loc,och2), j, och1]
        wsl = np.zeros((8, 15, NJ, 120), f32)
        nrow = min(nb, 8)
        wsl[0:nrow] = fc1_w4[:, :, a:a + nrow, :].transpose(2, 1, 3, 0)
        wslab = wsl.reshape(120, NJ, 120).astype(bf16)

        we = np.zeros((128, 1480), f32)
        we[:, 0:600] = w128.reshape(128, 600)
        we[0:5, 714:720] = w1fix        # M cols 114..119 of the [5,120] block
        we[0:120, 720:834] = s1m
        we[0:120, 834:954] = s2a
        we[0:15, 954:1074] = s2b
        we[0:114, 1074:1479] = w2.reshape(114, 405)

        m = {
            "we": we.astype(bf16),
            "bsmall": bsmall,
            "xs": x128.reshape(128, B * XC).astype(bf16),
            "xf": xf.reshape(5, B * 124).astype(bf16),
            "wslab": np.ascontiguousarray(wslab),
            "pack1": pack1,
        }
        in_maps.append(m)
    return in_maps


_NC_CACHE = None


def kernel(**inputs) -> np.ndarray:
    global _NC_CACHE
    if _NC_CACHE is None:
        _NC_CACHE = _build_nc()
    nc = _NC_CACHE
    in_maps = _prep_inputs(inputs)
    res = bass_utils.run_bass_kernel_spmd(
        nc, in_maps, core_ids=list(range(N_CORES)))
    return res.results[0]["out"]


# revision 21
# speedup vs baseline: 1.1598x; 1.0369x over previous
"""Trainium2 Bass kernel for nn_ClassicalHybridClassifier (v2).

Pipeline: conv1(5x5,s2) -> maxpool(2,s1) -> conv2(3x3,s2) -> maxpool(2,s1)
          -> fc1 [120,55815] -> fc2 -> fc3 -> qnn tanh stack -> RBF vs 8192
          train states -> [1,2] output.

Sharding: each of the 8 cores computes a horizontal band of the conv pipeline
(bands over the 61 pool2 output rows: 8,8,8,8,8,7,7,7) and the matching
contraction slice of fc1 (tensor-parallel over fc1's 55815 input dim). One
AllReduce of the [10,120] fc1 partials; the tiny tail (fc2/fc3/qnn/RBF over
all 8192 train states) is replicated on every core.

v2 vs v1:
- whole conv/fc1 path in bf16 (measured end-to-end rel err ~3e-3 vs 2e-2 gate)
- conv1 contraction packed into a single K=128 matmul group (c0,c1 full 43
  rows; c2 rows 0..41) plus a K=5 fixup matmul for the (c2,r42) row whose
  kx taps are materialized host-side as 5 shifted stream rows
- conv biases folded into the PSUM-eviction activation (no ones rows)
- fc1 single-bf16 (61 matmuls), j-major pool2 so fc1 follows immediately
- chunked, prioritized input DMAs (weights-early, 3 x chunks) on HWDGE so the
  PE starts within ~3us and stays p-state hot (warmup matmul train up front)
- dummy AllReduce early to absorb the CC engine's ~11us cold-start before the
  real AllReduce
- tail: PE broadcast of fs, RBF in inner-product form with exp(-|ts|^2)
  folded host-side into the classifier weights
"""

import numpy as np
import ml_dtypes

import concourse.bass as bass
import concourse.mybir as mybir
import concourse.tile as tile
from concourse import bass_utils, bacc

F32 = mybir.dt.float32
BF16 = mybir.dt.bfloat16
AF = mybir.ActivationFunctionType
ALU = mybir.AluOpType
AX = mybir.AxisListType

N_CORES = 8
BANDS = [(0, 8), (8, 16), (16, 24), (24, 32), (32, 40), (40, 47), (47, 54), (54, 61)]

B = 10          # batch
XR = 43         # x rows per core (padded)
XC = 252        # x cols incl 1+1 zero pad
C1R = 20        # conv1 out rows per core (padded)
P1R = 19        # pool1 rows per core (padded)
C2R = 9         # conv2 out rows per core (padded)
NJ = 61         # pool2 / fc1 spatial columns

C1_CHUNKS = [(0, 4), (4, 3), (7, 3)]     # conv1/pool1 image chunks (ni*124<=512)
C2_CHUNKS = [(0, 8), (8, 2)]             # conv2 image chunks (ni*62<=512)
J_BLOCKS = [(0, 31), (31, 30)]           # pool2/fc1 j blocks (B*jw<=512)


def _build_nc():
    nc = bacc.Bacc("TRN2", target_bir_lowering=False, debug=False,
                   num_devices=N_CORES)

    d = {}
    def din(name, shape, dt):
        d[name] = nc.dram_tensor(name, list(shape), dt, kind="ExternalInput").ap()

    din("we", (128, 1480), BF16)       # w1|w1fix|s1m|s2a|s2b|w2f
    din("bsmall", (128, 4), F32)       # conv biases
    din("xs", (128, B * XC), BF16)     # x band, K=128 partition layout
    din("xf", (5, B * 124), BF16)      # (c2,r42) kx-shifted fixup stream rows
    din("wslab", (120, NJ, 120), BF16) # fc1 weight slab
    din("pack1", (128, 576), F32)      # tail smalls

    out_d = nc.dram_tensor("out", [1, 2], F32, kind="ExternalOutput").ap()
    warm_d = nc.dram_tensor("warm", [1, 4], F32, kind="ExternalOutput").ap()

    with tile.TileContext(nc) as tc:
        with (
            tc.tile_pool(name="sb", bufs=1) as sb,
            tc.tile_pool(name="dr", bufs=1, space="DRAM") as dr,
        ):
            # ---- input tiles ----
            we_t = sb.tile([128, 1480], BF16)
            bs_t = sb.tile([128, 4], F32)
            xs_t = sb.tile([128, B * XC], BF16)
            xf_t = sb.tile([5, B * 124], BF16)
            wslab_t = sb.tile([120, NJ, 120], BF16)
            pack1_t = sb.tile([128, 576], F32)

            # ---- DMAs in, all on the sync/scalar HWDGE queues. The gpsimd
            # SWDGE rings are left untouched: SWDGE descriptor traffic delays
            # the CC engine's bring-up (its DMA_ADVANCE steps), which gates
            # when the AllReduce can run — and that, not the conv pipeline,
            # is the critical path. ----
            nc.sync.dma_start(bs_t[:], d["bsmall"][:])
            nc.scalar.dma_start(we_t[:, 0:720], d["we"][:, 0:720])
            nc.sync.dma_start(xs_t[:, 0:4 * XC], d["xs"][:, 0:4 * XC])
            nc.sync.dma_start(xf_t[:], d["xf"][:])
            nc.sync.dma_start(xs_t[:, 4 * XC:7 * XC], d["xs"][:, 4 * XC:7 * XC])
            nc.sync.dma_start(xs_t[:, 7 * XC:B * XC], d["xs"][:, 7 * XC:B * XC])
            nc.scalar.dma_start(we_t[:, 720:1480], d["we"][:, 720:1480])
            nc.scalar.dma_start(wslab_t[:, 0:31, :], d["wslab"][:, 0:31, :])
            nc.scalar.dma_start(wslab_t[:, 31:NJ, :], d["wslab"][:, 31:NJ, :])
            nc.scalar.dma_start(pack1_t[:], d["pack1"][:])

            x_r = xs_t[:].rearrange("p (i c) -> p i c", c=XC)
            xf_r = xf_t[:].rearrange("p (i c) -> p i c", c=124)
            w1_t = we_t[:, 0:600]                  # [128, 5*120]
            w1fix_t = we_t[0:5, 600:720]           # [5, 120] (cols 114:120 live)
            s1m_t = we_t[0:120, 720:834]           # [120, 114]
            s2a_t = we_t[0:120, 834:954]           # [120, 120]
            s2b_t = we_t[0:15, 954:1074]           # [15, 120]
            w2f_t = we_t[0:114, 1074:1479]         # [114, 3*135]
            b1 = bs_t[0:120, 0:1]
            b2a = bs_t[0:120, 1:2]
            b2b = bs_t[0:15, 2:3]

            small = {
                "fc1b": pack1_t[0:120, 0:1],
                "w2fcT": pack1_t[0:120, 1:85],
                "fc2b": pack1_t[0:84, 85:86],
                "w3fcT": pack1_t[0:84, 86:87],
                "b3vec": pack1_t[0:B, 87:88],
                "wq1T": pack1_t[0:B, 88:108],
                "wq2T": pack1_t[0:20, 108:113],
                "idt10": pack1_t[0:B, 113:123],
                "kclsb": pack1_t[0:1, 123:125],
                "ts2_r": pack1_t[:, 128:448].rearrange("p (a b) -> p a b", b=5),
                "acls_r": pack1_t[:, 448:576].rearrange("p (a b) -> p a b", b=64),
            }

            # ---- PE warmup (p-state ramp) during input DMA ----
            with tc.tile_pool(name="ps_w", bufs=1, space="PSUM") as ps_w:
                wsc = sb.tile([128, 512], BF16)
                nc.vector.memset(wsc[:], 0.0)
                wps = ps_w.tile([128, 512], F32)
                for i in range(10):
                    nc.tensor.matmul(wps[:], wsc[:, 0:128], wsc[:],
                                     start=(i == 0), stop=(i == 9))
                wout = sb.tile([1, 4], F32)
                nc.vector.tensor_copy(wout[:], wps[0:1, 0:4])
                nc.sync.dma_start(warm_d[:], wout[:])

            # ---- conv1 + pool1 ----
            Cs = sb.tile([120, B, 124], BF16)      # conv1 evict (relu+bias)
            Ch = sb.tile([120, B, 123], BF16)      # horizontal max
            V = sb.tile([114, B, 125], BF16)       # pool1 out
            nc.vector.memset(V[:, :, 0:1], 0.0)
            nc.vector.memset(V[:, :, 124:125], 0.0)

            with tc.tile_pool(name="ps_1", bufs=1, space="PSUM") as ps1:
                Cp = ps1.tile([120, 1536], F32)    # conv1 psum, 3 banks
                Sh = ps1.tile([114, 1536], F32)    # pool1 vertical-shift psum

                def conv1_chunk(ci, i0, ni):
                    n = ni * 124
                    for kx in range(5):
                        nc.tensor.matmul(
                            Cp[:, ci * 512: ci * 512 + n],
                            w1_t[:, kx * 120:(kx + 1) * 120],
                            x_r[:, i0:i0 + ni, kx:kx + 248:2],
                            start=(kx == 0), stop=False)
                    nc.tensor.matmul(
                        Cp[:, ci * 512: ci * 512 + n],
                        w1fix_t[:], xf_r[:, i0:i0 + ni, :],
                        start=False, stop=True)

                def conv1_evict(ci, i0, ni):
                    cv = Cp[:, ci * 512: ci * 512 + ni * 124].rearrange(
                        "p (i x) -> p i x", x=124)
                    nc.scalar.activation(Cs[:, i0:i0 + ni, :], cv, AF.Relu,
                                         bias=b1)
                    nc.vector.tensor_max(Ch[:, i0:i0 + ni, :],
                                         Cs[:, i0:i0 + ni, 0:123],
                                         Cs[:, i0:i0 + ni, 1:124])

                def pool1_chunk(ci, i0, ni):
                    n = ni * 123
                    nc.tensor.matmul(
                        Sh[:, ci * 512: ci * 512 + n],
                        s1m_t[:, 0:114], Ch[:, i0:i0 + ni, :],
                        start=True, stop=True)
                    sv = Sh[:, ci * 512: ci * 512 + n].rearrange(
                        "p (i x) -> p i x", x=123)
                    nc.vector.tensor_max(V[0:114, i0:i0 + ni, 1:124],
                                         Ch[0:114, i0:i0 + ni, :], sv)

                # PE order keeps the systolic array continuously busy
                conv1_chunk(0, 0, 4)
                conv1_evict(0, 0, 4)
                conv1_chunk(1, 4, 3)
                conv1_evict(1, 4, 3)
                pool1_chunk(0, 0, 4)
                conv1_chunk(2, 7, 3)
                conv1_evict(2, 7, 3)
                pool1_chunk(1, 4, 3)
                pool1_chunk(2, 7, 3)

            # ---- conv2 + pool2 + fc1 ----
            C2s_a = sb.tile([120, B, 62], BF16)
            C2s_b = sb.tile([15, B, 62], BF16)
            C2h_a = sb.tile([120, B, 61], BF16)
            C2h_b = sb.tile([15, B, 61], BF16)
            V2 = sb.tile([120, B, NJ], BF16)       # pool2 out
            fc1s = sb.tile([B, 120], F32)

            with tc.tile_pool(name="ps_2", bufs=1, space="PSUM") as ps2:
                C2a = ps2.tile([120, 1024], F32)
                C2b = ps2.tile([15, 1024], F32)
                Sh2 = ps2.tile([120, 1024], F32)
                fps = ps2.tile([B, 120], F32)

                def conv2_chunk(ci, i0, ni):
                    n = ni * 62
                    for grp, (cp, m0, m1) in enumerate(
                            ((C2a, 0, 120), (C2b, 120, 135))):
                        for kxp in range(3):
                            nc.tensor.matmul(
                                cp[:, ci * 512: ci * 512 + n],
                                w2f_t[:, kxp * 135 + m0: kxp * 135 + m1],
                                V[:, i0:i0 + ni, kxp:kxp + 123:2],
                                start=(kxp == 0), stop=(kxp == 2))

                def conv2_evict(ci, i0, ni):
                    for cp, cs, ch, bb in ((C2a, C2s_a, C2h_a, b2a),
                                           (C2b, C2s_b, C2h_b, b2b)):
                        cv = cp[:, ci * 512: ci * 512 + ni * 62].rearrange(
                            "p (i x) -> p i x", x=62)
                        nc.scalar.activation(cs[:, i0:i0 + ni, :], cv, AF.Relu,
                                             bias=bb)
                        nc.vector.tensor_max(ch[:, i0:i0 + ni, :],
                                             cs[:, i0:i0 + ni, 0:61],
                                             cs[:, i0:i0 + ni, 1:62])

                def pool2_block(bi, j0, jw):
                    n = jw * B
                    nc.tensor.matmul(
                        Sh2[:, bi * 512: bi * 512 + n],
                        s2a_t[:], C2h_a[:, :, j0:j0 + jw],
                        start=True, stop=False)
                    nc.tensor.matmul(
                        Sh2[:, bi * 512: bi * 512 + n],
                        s2b_t[:], C2h_b[:, :, j0:j0 + jw],
                        start=False, stop=True)
                    sv = Sh2[:, bi * 512: bi * 512 + n].rearrange(
                        "p (i j) -> p i j", j=jw)
                    nc.vector.tensor_max(V2[:, :, j0:j0 + jw],
                                         C2h_a[:, :, j0:j0 + jw], sv)

                conv2_chunk(0, 0, 8)
                conv2_evict(0, 0, 8)
                conv2_chunk(1, 8, 2)
                conv2_evict(1, 8, 2)
                pool2_block(0, 0, 31)
                pool2_block(1, 31, 30)

                for j in range(NJ):
                    nc.tensor.matmul(fps[:], V2[:, :, j], wslab_t[:, j, :],
                                     start=(j == 0), stop=(j == NJ - 1))
                nc.vector.tensor_copy(fc1s[:], fps[:])

            # ---- AllReduce fc1 partials ----
            arin = dr.tile([B, 120], F32)
            arout = dr.tile([B, 120], F32, addr_space="Shared")
            nc.sync.dma_start(arin[:], fc1s[:])
            nc.gpsimd.collective_compute(
                "AllReduce", ALU.add,
                replica_groups=[list(range(N_CORES))],
                ins=[arin.opt()], outs=[arout.opt()])
            h1post = sb.tile([B, 120], F32)
            nc.sync.dma_start(h1post[:], arout[:])

            # ---- tail (replicated) ----
            h1 = sb.tile([120, B], F32)
            h2 = sb.tile([84, B], F32)
            h10 = sb.tile([B, 1], F32)
            s1 = sb.tile([20, 1], F32)
            fs_row = sb.tile([1, 5], F32)
            fsb_sb = sb.tile([128, 5], F32)
            fs2 = sb.tile([128, 5], F32)
            negs2 = sb.tile([128, 1], F32)
            prodz = sb.tile([128, 64, 5], F32)
            z = sb.tile([128, 64], F32)
            kxv = sb.tile([128, 64], F32)
            pr = sb.tile([128, 2, 64], F32)
            krw = sb.tile([128, 2], F32)
            ones_row = sb.tile([1, 128], F32)
            ones_col = sb.tile([128, 1], F32)
            out_sb = sb.tile([1, 2], F32)
            nc.vector.memset(ones_row[:], 1.0)
            nc.vector.memset(ones_col[:], 1.0)

            with tc.tile_pool(name="ps_4", bufs=1, space="PSUM") as ps4:
                tp = ps4.tile([120, B], F32)
                nc.tensor.transpose(tp[:], h1post[:], small["idt10"][:])
                nc.scalar.activation(h1[:], tp[:], AF.Relu,
                                     bias=small["fc1b"][:])

                p2 = ps4.tile([84, B], F32)
                nc.tensor.matmul(p2[:], small["w2fcT"][:], h1[:],
                                 start=True, stop=True)
                nc.scalar.activation(h2[:], p2[:], AF.Relu,
                                     bias=small["fc2b"][:])

                p3 = ps4.tile([B, 1], F32)
                nc.tensor.matmul(p3[:], h2[:], small["w3fcT"][:],
                                 start=True, stop=True)
                nc.scalar.activation(h10[:], p3[:], AF.Identity,
                                     bias=small["b3vec"][:])

                p4 = ps4.tile([20, 1], F32)
                nc.tensor.matmul(p4[:], small["wq1T"][:], h10[:],
                                 start=True, stop=True)
                nc.scalar.activation(s1[:], p4[:], AF.Tanh)

                p5 = ps4.tile([1, 5], F32)
                nc.tensor.matmul(p5[:], s1[:], small["wq2T"][:],
                                 start=True, stop=True)
                nc.scalar.activation(fs_row[:], p5[:], AF.Tanh)

                # broadcast fs to 128 partitions on the PE
                fsb_ps = ps4.tile([128, 5], F32)
                nc.tensor.matmul(fsb_ps[:], ones_row[:], fs_row[:],
                                 start=True, stop=True)
                nc.scalar.activation(fsb_sb[:], fsb_ps[:], AF.Identity)

                # |fs|^2 -> negs2 ; z = 2<ts, fs> ; K = exp(z - |fs|^2)
                nc.vector.scalar_tensor_tensor(
                    fs2[:], fsb_sb[:], -1.0, fsb_sb[:],
                    op0=ALU.mult, op1=ALU.mult)
                nc.vector.reduce_sum(negs2[:], fs2[:], axis=AX.X)
                nc.vector.tensor_mul(
                    prodz[:], small["ts2_r"][:],
                    fsb_sb[:].unsqueeze(1).broadcast_to([128, 64, 5]))
                nc.vector.reduce_sum(z[:], prodz[:], axis=AX.X)
                nc.scalar.activation(kxv[:], z[:], AF.Exp, bias=negs2[:])
                nc.vector.tensor_mul(
                    pr[:], small["acls_r"][:],
                    kxv[:].unsqueeze(1).broadcast_to([128, 2, 64]))
                nc.vector.reduce_sum(krw[:], pr[:], axis=AX.X)

                p6 = ps4.tile([1, 2], F32)
                nc.tensor.matmul(p6[:], ones_col[:], krw[:],
                                 start=True, stop=True)
                nc.vector.tensor_add(out_sb[:], p6[:], small["kclsb"][:])

            nc.sync.dma_start(out_d[:], out_sb[:])

    nc.compile()
    return nc


def _prep_inputs(inputs):
    f32 = np.float32
    bf16 = ml_dtypes.bfloat16
    x = np.asarray(inputs["x"], f32)
    conv1_w = np.asarray(inputs["conv1_w"], f32)
    conv1_b = np.asarray(inputs["conv1_b"], f32)
    conv2_w = np.asarray(inputs["conv2_w"], f32)
    conv2_b = np.asarray(inputs["conv2_b"], f32)
    fc1_w = np.asarray(inputs["fc1_w"], f32)
    fc1_b = np.asarray(inputs["fc1_b"], f32)
    fc2_w = np.asarray(inputs["fc2_w"], f32)
    fc2_b = np.asarray(inputs["fc2_b"], f32)
    fc3_w = np.asarray(inputs["fc3_w"], f32)
    fc3_b = np.asarray(inputs["fc3_b"], f32)
    qnn_w1 = np.asarray(inputs["qnn_w1"], f32)
    qnn_w2 = np.asarray(inputs["qnn_w2"], f32)
    ts = np.asarray(inputs["train_states"], f32)
    kcls_w = np.asarray(inputs["kcls_w"], f32)
    kcls_b = np.asarray(inputs["kcls_b"], f32)

    pack1 = np.zeros((128, 576), f32)
    pack1[0:120, 0:1] = fc1_b.reshape(120, 1)
    pack1[0:120, 1:85] = fc2_w.T
    pack1[0:84, 85:86] = fc2_b.reshape(84, 1)
    pack1[0:84, 86:87] = fc3_w.T
    pack1[0:B, 87:88] = fc3_b[0]
    pack1[0:B, 88:108] = qnn_w1.T
    pack1[0:20, 108:113] = qnn_w2.T
    pack1[0:B, 113:123] = np.eye(B, dtype=f32)
    pack1[0:1, 123:125] = kcls_b.reshape(1, 2)
    pack1[:, 128:448] = (2.0 * ts).reshape(128, 320)
    acls = kcls_w * np.exp(-np.sum(ts * ts, axis=-1))[None, :]   # [2, 8192]
    pack1[:, 448:576] = acls.reshape(2, 128, 64).transpose(1, 0, 2).reshape(128, 128)

    bsmall = np.zeros((128, 4), f32)
    bsmall[0:120, 0] = np.tile(conv1_b, C1R)
    bsmall[0:120, 1] = np.tile(conv2_b, 8)
    bsmall[0:15, 2] = conv2_b

    fc1_w4 = fc1_w.reshape(120, 15, 61, 61)

    in_maps = []
    for a, b in BANDS:
        nb = b - a
        Y0 = 2 * a - 1          # conv1 row of y_loc 0 (also pool1 row of py_loc 0)
        X0 = 4 * a - 3          # x row of r_loc 0

        # x slab: K=128 partitions = c0 r0-42 | c1 r0-42 | c2 r0-41
        xsl = np.zeros((3, XR, B, XC), f32)
        r_lo, r_hi = max(0, X0), min(250, X0 + XR)
        xsl[:, r_lo - X0: r_hi - X0, :, 1:251] = (
            x[:, :, r_lo:r_hi, :].transpose(1, 2, 0, 3))
        x128 = np.concatenate([xsl[0], xsl[1], xsl[2][0:42]], axis=0)
        xf = np.zeros((5, B, 124), f32)
        for kx in range(5):
            xf[kx] = xsl[2][42][:, kx:kx + 248:2]

        # conv1 banded weights: K=(c, r_loc), M=(y_loc, och), per kx
        w1 = np.zeros((3, XR, 5, 120), f32)
        for y_loc in range(C1R):
            y = Y0 + y_loc
            if not (0 <= y <= 123):
                continue
            for ky in range(5):
                r_loc = 2 * y_loc + ky
                if r_loc >= XR:
                    continue
                for c in range(3):
                    w1[c, r_loc, :, y_loc * 6: y_loc * 6 + 6] = \
                        conv1_w[:, c, ky, :].T
        w128 = np.concatenate([w1[0], w1[1], w1[2][0:42]], axis=0)  # [128,5,120]
        w1fix = np.zeros((5, 6), f32)
        if 0 <= Y0 + 19 <= 123:
            w1fix[:, :] = conv1_w[:, 2, 4, :].T     # (c2, ky=4) taps, [kx, och]

        # conv2 banded weights: K=(py_loc, ich), M=(i2_loc, och2)
        w2 = np.zeros((114, 3, 135), f32)
        for i2_loc in range(C2R):
            i2 = a + i2_loc
            if i2 > 61:
                continue
            for kyp in range(3):
                py_loc = 2 * i2_loc + kyp
                py = Y0 + py_loc
                if py_loc >= P1R or not (0 <= py <= 122):
                    continue
                for ich in range(6):
                    q = py_loc * 6 + ich
                    m0 = i2_loc * 15
                    w2[q, :, m0:m0 + 15] = conv2_w[:, ich, kyp, :].T

        # partition-shift matrices
        s1m = np.zeros((120, 114), f32)
        for m in range(114):
            s1m[m + 6, m] = 1.0
        s2a = np.zeros((120, 120), f32)
        s2b = np.zeros((15, 120), f32)
        for m in range(105):
            s2a[m + 15, m] = 1.0
        for m in range(105, 120):
            s2b[m - 105, m] = 1.0

        # fc1 weight slab: [p=(i2_loc,och2), j, och1]
        wsl = np.zeros((8, 15, NJ, 120), f32)
        nrow = min(nb, 8)
        wsl[0:nrow] = fc1_w4[:, :, a:a + nrow, :].transpose(2, 1, 3, 0)
        wslab = wsl.reshape(120, NJ, 120).astype(bf16)

        we = np.zeros((128, 1480), f32)
        we[:, 0:600] = w128.reshape(128, 600)
        we[0:5, 714:720] = w1fix        # M cols 114..119 of the [5,120] block
        we[0:120, 720:834] = s1m
        we[0:120, 834:954] = s2a
        we[0:15, 954:1074] = s2b
        we[0:114, 1074:1479] = w2.reshape(114, 405)

        m = {
            "we": we.astype(bf16),
            "bsmall": bsmall,
            "xs": x128.reshape(128, B * XC).astype(bf16),
            "xf": xf.reshape(5, B * 124).astype(bf16),
            "wslab": np.ascontiguousarray(wslab),
            "pack1": pack1,
        }
        in_maps.append(m)
    return in_maps


_NC_CACHE = None


def kernel(**inputs) -> np.ndarray:
    global _NC_CACHE
    if _NC_CACHE is None:
        _NC_CACHE = _build_nc()
    nc = _NC_CACHE
    in_maps = _prep_inputs(inputs)
    res = bass_utils.run_bass_kernel_spmd(
        nc, in_maps, core_ids=list(range(N_CORES)))
    return res.results[0]["out"]


# revision 24
# speedup vs baseline: 1.2209x; 1.0527x over previous
"""Trainium2 Bass kernel for nn_ClassicalHybridClassifier (v2).

Pipeline: conv1(5x5,s2) -> maxpool(2,s1) -> conv2(3x3,s2) -> maxpool(2,s1)
          -> fc1 [120,55815] -> fc2 -> fc3 -> qnn tanh stack -> RBF vs 8192
          train states -> [1,2] output.

Sharding: each of the 8 cores computes a horizontal band of the conv pipeline
(bands over the 61 pool2 output rows: 8,8,8,8,8,7,7,7) and the matching
contraction slice of fc1 (tensor-parallel over fc1's 55815 input dim). One
AllReduce of the [10,120] fc1 partials; the tiny tail (fc2/fc3/qnn/RBF over
all 8192 train states) is replicated on every core.

v2 vs v1:
- whole conv/fc1 path in bf16 (measured end-to-end rel err ~3e-3 vs 2e-2 gate)
- conv1 contraction packed into a single K=128 matmul group (c0,c1 full 43
  rows; c2 rows 0..41) plus a K=5 fixup matmul for the (c2,r42) row whose
  kx taps are materialized host-side as 5 shifted stream rows
- conv biases folded into the PSUM-eviction activation (no ones rows)
- fc1 single-bf16 (61 matmuls), j-major pool2 so fc1 follows immediately
- chunked, prioritized input DMAs (weights-early, 3 x chunks) on HWDGE so the
  PE starts within ~3us and stays p-state hot (warmup matmul train up front)
- dummy AllReduce early to absorb the CC engine's ~11us cold-start before the
  real AllReduce
- tail: PE broadcast of fs, RBF in inner-product form with exp(-|ts|^2)
  folded host-side into the classifier weights
"""

import numpy as np
import ml_dtypes

import concourse.bass as bass
import concourse.mybir as mybir
import concourse.tile as tile
from concourse import bass_utils, bacc

F32 = mybir.dt.float32
BF16 = mybir.dt.bfloat16
AF = mybir.ActivationFunctionType
ALU = mybir.AluOpType
AX = mybir.AxisListType

N_CORES = 8
BANDS = [(0, 8), (8, 16), (16, 24), (24, 32), (32, 40), (40, 47), (47, 54), (54, 61)]

B = 10          # batch
XR = 43         # x rows per core (padded)
XC = 252        # x cols incl 1+1 zero pad
C1R = 20        # conv1 out rows per core (padded)
P1R = 19        # pool1 rows per core (padded)
C2R = 9         # conv2 out rows per core (padded)
NJ = 61         # pool2 / fc1 spatial columns

C1_CHUNKS = [(0, 4), (4, 3), (7, 3)]     # conv1/pool1 image chunks (ni*124<=512)
C2_CHUNKS = [(0, 8), (8, 2)]             # conv2 image chunks (ni*62<=512)
J_BLOCKS = [(0, 31), (31, 30)]           # pool2/fc1 j blocks (B*jw<=512)


def _build_nc():
    nc = bacc.Bacc("TRN2", target_bir_lowering=False, debug=False,
                   num_devices=N_CORES)

    d = {}
    def din(name, shape, dt):
        d[name] = nc.dram_tensor(name, list(shape), dt, kind="ExternalInput").ap()

    din("we", (128, 1480), BF16)       # w1|w1fix|s1m|s2a|s2b|w2f
    din("bsmall", (128, 4), F32)       # conv biases
    din("xs", (128, B * XC), BF16)     # x band, K=128 partition layout
    din("xf", (5, B * 124), BF16)      # (c2,r42) kx-shifted fixup stream rows
    din("wslab", (120, NJ, 120), BF16) # fc1 weight slab
    din("pack1", (128, 576), F32)      # tail smalls

    out_d = nc.dram_tensor("out", [1, 2], F32, kind="ExternalOutput").ap()
    warm_d = nc.dram_tensor("warm", [1, 4], F32, kind="ExternalOutput").ap()

    with tile.TileContext(nc) as tc:
        with (
            tc.tile_pool(name="sb", bufs=1) as sb,
            tc.tile_pool(name="dr", bufs=1, space="DRAM") as dr,
        ):
            # ---- input tiles ----
            we_t = sb.tile([128, 1480], BF16)
            bs_t = sb.tile([128, 4], F32)
            xs_t = sb.tile([128, B * XC], BF16)
            xf_t = sb.tile([5, B * 124], BF16)
            wslab_t = sb.tile([120, NJ, 120], BF16)
            pack1_t = sb.tile([128, 576], F32)

            # ---- DMAs in, all on the sync/scalar HWDGE queues. The gpsimd
            # SWDGE rings are left untouched: SWDGE descriptor traffic delays
            # the CC engine's bring-up (its DMA_ADVANCE steps), which gates
            # when the AllReduce can run — and that, not the conv pipeline,
            # is the critical path. ----
            nc.sync.dma_start(bs_t[:], d["bsmall"][:])
            nc.scalar.dma_start(we_t[:, 0:720], d["we"][:, 0:720])
            nc.sync.dma_start(xs_t[:, 0:4 * XC], d["xs"][:, 0:4 * XC])
            nc.sync.dma_start(xf_t[:], d["xf"][:])
            nc.sync.dma_start(xs_t[:, 4 * XC:7 * XC], d["xs"][:, 4 * XC:7 * XC])
            nc.sync.dma_start(xs_t[:, 7 * XC:B * XC], d["xs"][:, 7 * XC:B * XC])
            nc.scalar.dma_start(we_t[:, 720:1480], d["we"][:, 720:1480])
            nc.scalar.dma_start(wslab_t[:, 0:31, :], d["wslab"][:, 0:31, :])
            nc.scalar.dma_start(wslab_t[:, 31:NJ, :], d["wslab"][:, 31:NJ, :])
            nc.scalar.dma_start(pack1_t[:], d["pack1"][:])

            x_r = xs_t[:].rearrange("p (i c) -> p i c", c=XC)
            xf_r = xf_t[:].rearrange("p (i c) -> p i c", c=124)
            w1_t = we_t[:, 0:600]                  # [128, 5*120]
            w1fix_t = we_t[0:5, 600:720]           # [5, 120] (cols 114:120 live)
            s1m_t = we_t[0:120, 720:834]           # [120, 114]
            s2a_t = we_t[0:120, 834:954]           # [120, 120]
            s2b_t = we_t[0:15, 954:1074]           # [15, 120]
            w2f_t = we_t[0:114, 1074:1479]         # [114, 3*135]
            b1 = bs_t[0:120, 0:1]
            b2a = bs_t[0:120, 1:2]
            b2b = bs_t[0:15, 2:3]

            small = {
                "fc1b": pack1_t[0:120, 0:1],
                "w2fcT": pack1_t[0:120, 1:85],
                "fc2b": pack1_t[0:84, 85:86],
                "w3fcT": pack1_t[0:84, 86:87],
                "b3vec": pack1_t[0:B, 87:88],
                "wq1T": pack1_t[0:B, 88:108],
                "wq2T": pack1_t[0:20, 108:113],
                "idt10": pack1_t[0:B, 113:123],
                "kclsb": pack1_t[0:1, 123:125],
                "ts2_r": pack1_t[:, 128:448].rearrange("p (a b) -> p a b", b=5),
                "acls_r": pack1_t[:, 448:576].rearrange("p (a b) -> p a b", b=64),
            }

            # ---- PE warmup (p-state ramp) during input DMA ----
            with tc.tile_pool(name="ps_w", bufs=1, space="PSUM") as ps_w:
                wsc = sb.tile([128, 512], BF16)
                nc.vector.memset(wsc[:], 0.0)
                wps = ps_w.tile([128, 512], F32)
                for i in range(10):
                    nc.tensor.matmul(wps[:], wsc[:, 0:128], wsc[:],
                                     start=(i == 0), stop=(i == 9))
                wout = sb.tile([1, 4], F32)
                nc.vector.tensor_copy(wout[:], wps[0:1, 0:4])
                nc.sync.dma_start(warm_d[:], wout[:])

            # ---- conv1 + pool1 ----
            Cs = sb.tile([120, B, 124], BF16)      # conv1 evict (relu+bias)
            Ch = sb.tile([120, B, 123], BF16)      # horizontal max
            V = sb.tile([114, B, 125], BF16)       # pool1 out
            nc.vector.memset(V[:, :, 0:1], 0.0)
            nc.vector.memset(V[:, :, 124:125], 0.0)

            with tc.tile_pool(name="ps_1", bufs=1, space="PSUM") as ps1:
                Cp = ps1.tile([120, 1536], F32)    # conv1 psum, 3 banks
                Sh = ps1.tile([114, 1536], F32)    # pool1 vertical-shift psum

                def conv1_chunk(ci, i0, ni):
                    n = ni * 124
                    for kx in range(5):
                        nc.tensor.matmul(
                            Cp[:, ci * 512: ci * 512 + n],
                            w1_t[:, kx * 120:(kx + 1) * 120],
                            x_r[:, i0:i0 + ni, kx:kx + 248:2],
                            start=(kx == 0), stop=False)
                    nc.tensor.matmul(
                        Cp[:, ci * 512: ci * 512 + n],
                        w1fix_t[:], xf_r[:, i0:i0 + ni, :],
                        start=False, stop=True)

                def conv1_evict(ci, i0, ni):
                    cv = Cp[:, ci * 512: ci * 512 + ni * 124].rearrange(
                        "p (i x) -> p i x", x=124)
                    nc.scalar.activation(Cs[:, i0:i0 + ni, :], cv, AF.Relu,
                                         bias=b1)
                    nc.vector.tensor_max(Ch[:, i0:i0 + ni, :],
                                         Cs[:, i0:i0 + ni, 0:123],
                                         Cs[:, i0:i0 + ni, 1:124])

                def pool1_chunk(ci, i0, ni):
                    n = ni * 123
                    nc.tensor.matmul(
                        Sh[:, ci * 512: ci * 512 + n],
                        s1m_t[:, 0:114], Ch[:, i0:i0 + ni, :],
                        start=True, stop=True)
                    sv = Sh[:, ci * 512: ci * 512 + n].rearrange(
                        "p (i x) -> p i x", x=123)
                    nc.vector.tensor_max(V[0:114, i0:i0 + ni, 1:124],
                                         Ch[0:114, i0:i0 + ni, :], sv)

                # PE order keeps the systolic array continuously busy
                conv1_chunk(0, 0, 4)
                conv1_evict(0, 0, 4)
                conv1_chunk(1, 4, 3)
                conv1_evict(1, 4, 3)
                pool1_chunk(0, 0, 4)
                conv1_chunk(2, 7, 3)
                conv1_evict(2, 7, 3)
                pool1_chunk(1, 4, 3)
                pool1_chunk(2, 7, 3)

            # ---- conv2 + pool2 + fc1 ----
            C2s_a = sb.tile([120, B, 62], BF16)
            C2s_b = sb.tile([15, B, 62], BF16)
            C2h_a = sb.tile([120, B, 61], BF16)
            C2h_b = sb.tile([15, B, 61], BF16)
            V2 = sb.tile([120, B, NJ], BF16)       # pool2 out
            fc1s = sb.tile([B, 120], F32)
            fc1sT = sb.tile([120, B], F32)

            with tc.tile_pool(name="ps_2", bufs=1, space="PSUM") as ps2:
                C2a = ps2.tile([120, 1024], F32)
                C2b = ps2.tile([15, 1024], F32)
                Sh2 = ps2.tile([120, 1024], F32)
                fps = ps2.tile([B, 120], F32)

                def conv2_chunk(ci, i0, ni):
                    n = ni * 62
                    for grp, (cp, m0, m1) in enumerate(
                            ((C2a, 0, 120), (C2b, 120, 135))):
                        for kxp in range(3):
                            nc.tensor.matmul(
                                cp[:, ci * 512: ci * 512 + n],
                                w2f_t[:, kxp * 135 + m0: kxp * 135 + m1],
                                V[:, i0:i0 + ni, kxp:kxp + 123:2],
                                start=(kxp == 0), stop=(kxp == 2))

                def conv2_evict(ci, i0, ni):
                    for cp, cs, ch, bb in ((C2a, C2s_a, C2h_a, b2a),
                                           (C2b, C2s_b, C2h_b, b2b)):
                        cv = cp[:, ci * 512: ci * 512 + ni * 62].rearrange(
                            "p (i x) -> p i x", x=62)
                        nc.scalar.activation(cs[:, i0:i0 + ni, :], cv, AF.Relu,
                                             bias=bb)
                        nc.vector.tensor_max(ch[:, i0:i0 + ni, :],
                                             cs[:, i0:i0 + ni, 0:61],
                                             cs[:, i0:i0 + ni, 1:62])

                def pool2_block(bi, j0, jw):
                    n = jw * B
                    nc.tensor.matmul(
                        Sh2[:, bi * 512: bi * 512 + n],
                        s2a_t[:], C2h_a[:, :, j0:j0 + jw],
                        start=True, stop=False)
                    nc.tensor.matmul(
                        Sh2[:, bi * 512: bi * 512 + n],
                        s2b_t[:], C2h_b[:, :, j0:j0 + jw],
                        start=False, stop=True)
                    sv = Sh2[:, bi * 512: bi * 512 + n].rearrange(
                        "p (i j) -> p i j", j=jw)
                    nc.vector.tensor_max(V2[:, :, j0:j0 + jw],
                                         C2h_a[:, :, j0:j0 + jw], sv)

                conv2_chunk(0, 0, 8)
                conv2_evict(0, 0, 8)
                conv2_chunk(1, 8, 2)
                conv2_evict(1, 8, 2)
                pool2_block(0, 0, 31)
                pool2_block(1, 31, 30)

                for j in range(NJ):
                    nc.tensor.matmul(fps[:], V2[:, :, j], wslab_t[:, j, :],
                                     start=(j == 0), stop=(j == NJ - 1))
                nc.vector.tensor_copy(fc1s[:], fps[:])

                # transpose to [120, B] pre-collective (this path has slack;
                # the post-collective tail is latency-critical)
                tp0 = ps2.tile([120, B], F32)
                nc.tensor.transpose(tp0[:], fc1s[:], small["idt10"][:])
                nc.vector.tensor_copy(fc1sT[:], tp0[:])

            # ---- AllReduce fc1 partials ----
            arin = dr.tile([120, B], F32)
            arout = dr.tile([120, B], F32, addr_space="Shared")
            nc.sync.dma_start(arin[:], fc1sT[:])
            nc.gpsimd.collective_compute(
                "AllReduce", ALU.add,
                replica_groups=[list(range(N_CORES))],
                ins=[arin.opt()], outs=[arout.opt()])
            h1post = sb.tile([120, B], F32)
            nc.sync.dma_start(h1post[:], arout[:])

            # ---- tail (replicated) ----
            h1 = sb.tile([120, B], F32)
            h2 = sb.tile([84, B], F32)
            h10 = sb.tile([B, 1], F32)
            s1 = sb.tile([20, 1], F32)
            fs_row = sb.tile([1, 5], F32)
            fsb_sb = sb.tile([128, 5], F32)
            fs2 = sb.tile([128, 5], F32)
            negs2 = sb.tile([128, 1], F32)
            prodz = sb.tile([128, 64, 5], F32)
            z = sb.tile([128, 64], F32)
            kxv = sb.tile([128, 64], F32)
            pr = sb.tile([128, 2, 64], F32)
            krw = sb.tile([128, 2], F32)
            ones_row = sb.tile([1, 128], F32)
            ones_col = sb.tile([128, 1], F32)
            out_sb = sb.tile([1, 2], F32)
            nc.vector.memset(ones_row[:], 1.0)
            nc.vector.memset(ones_col[:], 1.0)

            with tc.tile_pool(name="ps_4", bufs=1, space="PSUM") as ps4:
                nc.scalar.activation(h1[:], h1post[:], AF.Relu,
                                     bias=small["fc1b"][:])

                p2 = ps4.tile([84, B], F32)
                nc.tensor.matmul(p2[:], small["w2fcT"][:], h1[:],
                                 start=True, stop=True)
                nc.scalar.activation(h2[:], p2[:], AF.Relu,
                                     bias=small["fc2b"][:])

                p3 = ps4.tile([B, 1], F32)
                nc.tensor.matmul(p3[:], h2[:], small["w3fcT"][:],
                                 start=True, stop=True)
                nc.scalar.activation(h10[:], p3[:], AF.Identity,
                                     bias=small["b3vec"][:])

                p4 = ps4.tile([20, 1], F32)
                nc.tensor.matmul(p4[:], small["wq1T"][:], h10[:],
                                 start=True, stop=True)
                nc.scalar.activation(s1[:], p4[:], AF.Tanh)

                p5 = ps4.tile([1, 5], F32)
                nc.tensor.matmul(p5[:], s1[:], small["wq2T"][:],
                                 start=True, stop=True)
                nc.scalar.activation(fs_row[:], p5[:], AF.Tanh)

                # broadcast fs to 128 partitions on the PE
                fsb_ps = ps4.tile([128, 5], F32)
                nc.tensor.matmul(fsb_ps[:], ones_row[:], fs_row[:],
                                 start=True, stop=True)
                nc.scalar.activation(fsb_sb[:], fsb_ps[:], AF.Identity)

                # |fs|^2 -> negs2 ; z = 2<ts, fs> ; K = exp(z - |fs|^2)
                nc.vector.scalar_tensor_tensor(
                    fs2[:], fsb_sb[:], -1.0, fsb_sb[:],
                    op0=ALU.mult, op1=ALU.mult)
                nc.vector.reduce_sum(negs2[:], fs2[:], axis=AX.X)
                nc.vector.tensor_mul(
                    prodz[:], small["ts2_r"][:],
                    fsb_sb[:].unsqueeze(1).broadcast_to([128, 64, 5]))
                nc.vector.reduce_sum(z[:], prodz[:], axis=AX.X)
                nc.scalar.activation(kxv[:], z[:], AF.Exp, bias=negs2[:])
                nc.vector.tensor_mul(
                    pr[:], small["acls_r"][:],
                    kxv[:].unsqueeze(1).broadcast_to([128, 2, 64]))
                nc.vector.reduce_sum(krw[:], pr[:], axis=AX.X)

                p6 = ps4.tile([1, 2], F32)
                nc.tensor.matmul(p6[:], ones_col[:], krw[:],
                                 start=True, stop=True)
                nc.vector.tensor_add(out_sb[:], p6[:], small["kclsb"][:])

            nc.sync.dma_start(out_d[:], out_sb[:])

    nc.compile()
    return nc


def _prep_inputs(inputs):
    f32 = np.float32
    bf16 = ml_dtypes.bfloat16
    x = np.asarray(inputs["x"], f32)
    conv1_w = np.asarray(inputs["conv1_w"], f32)
    conv1_b = np.asarray(inputs["conv1_b"], f32)
    conv2_w = np.asarray(inputs["conv2_w"], f32)
    conv2_b = np.asarray(inputs["conv2_b"], f32)
    fc1_w = np.asarray(inputs["fc1_w"], f32)
    fc1_b = np.asarray(inputs["fc1_b"], f32)
    fc2_w = np.asarray(inputs["fc2_w"], f32)
    fc2_b = np.asarray(inputs["fc2_b"], f32)
    fc3_w = np.asarray(inputs["fc3_w"], f32)
    fc3_b = np.asarray(inputs["fc3_b"], f32)
    qnn_w1 = np.asarray(inputs["qnn_w1"], f32)
    qnn_w2 = np.asarray(inputs["qnn_w2"], f32)
    ts = np.asarray(inputs["train_states"], f32)
    kcls_w = np.asarray(inputs["kcls_w"], f32)
    kcls_b = np.asarray(inputs["kcls_b"], f32)

    pack1 = np.zeros((128, 576), f32)
    pack1[0:120, 0:1] = fc1_b.reshape(120, 1)
    pack1[0:120, 1:85] = fc2_w.T
    pack1[0:84, 85:86] = fc2_b.reshape(84, 1)
    pack1[0:84, 86:87] = fc3_w.T
    pack1[0:B, 87:88] = fc3_b[0]
    pack1[0:B, 88:108] = qnn_w1.T
    pack1[0:20, 108:113] = qnn_w2.T
    pack1[0:B, 113:123] = np.eye(B, dtype=f32)
    pack1[0:1, 123:125] = kcls_b.reshape(1, 2)
    pack1[:, 128:448] = (2.0 * ts).reshape(128, 320)
    acls = kcls_w * np.exp(-np.sum(ts * ts, axis=-1))[None, :]   # [2, 8192]
    pack1[:, 448:576] = acls.reshape(2, 128, 64).transpose(1, 0, 2).reshape(128, 128)

    bsmall = np.zeros((128, 4), f32)
    bsmall[0:120, 0] = np.tile(conv1_b, C1R)
    bsmall[0:120, 1] = np.tile(conv2_b, 8)
    bsmall[0:15, 2] = conv2_b

    fc1_w4 = fc1_w.reshape(120, 15, 61, 61)

    in_maps = []
    for a, b in BANDS:
        nb = b - a
        Y0 = 2 * a - 1          # conv1 row of y_loc 0 (also pool1 row of py_loc 0)
        X0 = 4 * a - 3          # x row of r_loc 0

        # x slab: K=128 partitions = c0 r0-42 | c1 r0-42 | c2 r0-41
        xsl = np.zeros((3, XR, B, XC), f32)
        r_lo, r_hi = max(0, X0), min(250, X0 + XR)
        xsl[:, r_lo - X0: r_hi - X0, :, 1:251] = (
            x[:, :, r_lo:r_hi, :].transpose(1, 2, 0, 3))
        x128 = np.concatenate([xsl[0], xsl[1], xsl[2][0:42]], axis=0)
        xf = np.zeros((5, B, 124), f32)
        for kx in range(5):
            xf[kx] = xsl[2][42][:, kx:kx + 248:2]

        # conv1 banded weights: K=(c, r_loc), M=(y_loc, och), per kx
        w1 = np.zeros((3, XR, 5, 120), f32)
        for y_loc in range(C1R):
            y = Y0 + y_loc
            if not (0 <= y <= 123):
                continue
            for ky in range(5):
                r_loc = 2 * y_loc + ky
                if r_loc >= XR:
                    continue
                for c in range(3):
                    w1[c, r_loc, :, y_loc * 6: y_loc * 6 + 6] = \
                        conv1_w[:, c, ky, :].T
        w128 = np.concatenate([w1[0], w1[1], w1[2][0:42]], axis=0)  # [128,5,120]
        w1fix = np.zeros((5, 6), f32)
        if 0 <= Y0 + 19 <= 123:
            w1fix[:, :] = conv1_w[:, 2, 4, :].T     # (c2, ky=4) taps, [kx, och]

        # conv2 banded weights: K=(py_loc, ich), M=(i2_loc, och2)
        w2 = np.zeros((114, 3, 135), f32)
        for i2_loc in range(C2R):
            i2 = a + i2_loc
            if i2 > 61:
                continue
            for kyp in range(3):
                py_loc = 2 * i2_loc + kyp
                py = Y0 + py_loc
                if py_loc >= P1R or not (0 <= py <= 122):
                    continue
                for ich in range(6):
                    q = py_loc * 6 + ich
                    m0 = i2_loc * 15
                    w2[q, :, m0:m0 + 15] = conv2_w[:, ich, kyp, :].T

        # partition-shift matrices
        s1m = np.zeros((120, 114), f32)
        for m in range(114):
            s1m[m + 6, m] = 1.0
        s2a = np.zeros((120, 120), f32)
        s2b = np.zeros((15, 120), f32)
        for m in range(105):
            s2a[m + 15, m] = 1.0
        for m in range(105, 120):
            s2b[m - 105, m] = 1.0

        # fc1 weight slab: [p=(i2_loc,och2), j, och1]
        wsl = np.zeros((8, 15, NJ, 120), f32)
        nrow = min(nb, 8)
        wsl[0:nrow] = fc1_w4[:, :, a:a + nrow, :].transpose(2, 1, 3, 0)
        wslab = wsl.reshape(120, NJ, 120).astype(bf16)

        we = np.zeros((128, 1480), f32)
        we[:, 0:600] = w128.reshape(128, 600)
        we[0:5, 714:720] = w1fix        # M cols 114..119 of the [5,120] block
        we[0:120, 720:834] = s1m
        we[0:120, 834:954] = s2a
        we[0:15, 954:1074] = s2b
        we[0:114, 1074:1479] = w2.reshape(114, 405)

        m = {
            "we": we.astype(bf16),
            "bsmall": bsmall,
            "xs": x128.reshape(128, B * XC).astype(bf16),
            "xf": xf.reshape(5, B * 124).astype(bf16),
            "wslab": np.ascontiguousarray(wslab),
            "pack1": pack1,
        }
        in_maps.append(m)
    return in_maps


_NC_CACHE = None


def kernel(**inputs) -> np.ndarray:
    global _NC_CACHE
    if _NC_CACHE is None:
        _NC_CACHE = _build_nc()
    nc = _NC_CACHE
    in_maps = _prep_inputs(inputs)
    res = bass_utils.run_bass_kernel_spmd(
        nc, in_maps, core_ids=list(range(N_CORES)))
    return res.results[0]["out"]
